# revision 21
# baseline (speedup 1.0000x reference)
"""Trainium2 Bass kernel for nn_NeRF_MLP_Compose (MoE-routed NeRF MLP).

Strategy (v3):
  - Host-side MoE dispatch: expert e's rows split across cores 2e, 2e+1;
    each core runs ONE expert dense over CAP=8704 rows and holds only that
    expert's weights (bf16).
  - Chunk-major DRAM layouts ([128, 68, ...]) so every x/d/out DMA is a
    contiguous 2D transfer (no strided descriptor storms).
  - Front-end (row-major) cut to ~5 ALU ops via a mod-1 range reduction:
      t = x' * 2^(i-1)  (turns);  u = t mod 1;  w = |u - 1/2|
      sin(2*pi*t) = Sin(pi - 2*pi*u);  cos(2*pi*t) = Sin(2*pi*w - pi/2)
    u/w are fp16 (fast DVE modes); one Sin-table ACT per half.
  - R=1024-row tiles (8 full + 1 half): z tiles are [128, 1024] fp32
    2-bank PSUM tiles, halving op/semaphore counts vs 512-row tiles.
  - MLP feature-major bf16: l0 K=96 (ones-row bias), hidden K=256 in
    2x128 chunks, out M=64.  Residual h' = s*t + h as two DVE ops
    (tensor_scalar 4x + tensor_tensor 2x).
  - Relus split ACT(5)/GPSIMD(3); transposes+input DMA on sync queue;
    weights on scalar queue; output stores on gpsimd pseudo-DMA.
"""
import sys
for _p in ("/opt/trn_rl_repo", "/root/.axon_site/_ro/trn_rl_repo"):
    if _p not in sys.path:
        sys.path.insert(0, _p)

import numpy as np
import ml_dtypes

N = 65536
E = 4            # experts
NCORE = 8
CAP = 8704       # rows per core
CG = 68          # 128-row chunks per core
NT = 8           # full 1024-row tiles; +1 half tile
C = 8            # 128-row chunks per full tile
R = 1024         # rows per full tile
NUM_FREQS = 10
HID = 256
DOUT = 64
NL = 4
PI = float(np.float32(np.pi))
TWO_PI = float(np.float32(2 * np.pi))
HALF_PI = float(np.float32(0.5 * np.pi))
MAGIC_C = float(np.float32(1.5 * 2 ** 23))

_compiled = {}
RUN_KWARGS = {}    # test.py may set e.g. {"trace": True}
LAST_RESULT = []   # test.py reads the BassKernelResults appended here


def _build_program():
    import concourse.bass as bass
    from concourse import bacc
    import concourse.mybir as mybir
    import concourse.tile as tile

    F32 = mybir.dt.float32
    F16 = mybir.dt.float16
    U16 = mybir.dt.uint16
    BF16 = mybir.dt.bfloat16
    P = 128
    ALU = mybir.AluOpType
    ACTF = mybir.ActivationFunctionType

    nc = bacc.Bacc("TRN2", target_bir_lowering=False, debug=False)

    # ---- DRAM I/O (per core; one expert's weights) ----
    xd_d = nc.dram_tensor("xd_cm", [P, CG, 5], F32, kind="ExternalInput").ap()
    fr_d = nc.dram_tensor("fr10", [NUM_FREQS], F32, kind="ExternalInput").ap()
    sc_d = nc.dram_tensor("scal3", [NL - 1], F32, kind="ExternalInput").ap()
    w0_d = nc.dram_tensor("w0", [P, HID], BF16, kind="ExternalInput").ap()
    wh_d = nc.dram_tensor("wh", [P, NL - 1, 2, 2, P], BF16,
                          kind="ExternalInput").ap()
    wo_d = nc.dram_tensor("wo", [P, 2, DOUT], BF16, kind="ExternalInput").ap()
    wos_d = nc.dram_tensor("wos", [P, 2, DOUT], BF16, kind="ExternalInput").ap()
    bh_d = nc.dram_tensor("bhr", [P, NL - 1, 2], F32, kind="ExternalInput").ap()
    bo_d = nc.dram_tensor("bor", [DOUT, 1], F32, kind="ExternalInput").ap()
    out_d = nc.dram_tensor("out_cm", [P, CG, DOUT], F32,
                           kind="ExternalOutput").ap()

    # tiles: 8 full (C=8 chunks) + 1 half (4 chunks)
    tiles = [(t, 8 * t, 8) for t in range(NT)] + [(NT, 64, 4)]

    with tile.TileContext(nc) as tc:
        with tc.tile_pool(name="const", bufs=1) as cpool, \
             tc.tile_pool(name="fr", bufs=4) as fpool, \
             tc.tile_pool(name="act", bufs=3) as apool, \
             tc.tile_pool(name="psz", bufs=3, space="PSUM") as zpool, \
             tc.tile_pool(name="pso", bufs=1, space="PSUM") as opool:

            # ---- front-end constants first ----
            scl = cpool.tile([P, NL - 1], F32)
            nc.sync.dma_start(
                out=scl,
                in_=bass.AP(tensor=sc_d.tensor, offset=0,
                            ap=[[0, P], [1, NL - 1]]))
            fr = cpool.tile([P, NUM_FREQS], F32)
            nc.sync.dma_start(
                out=fr,
                in_=bass.AP(tensor=fr_d.tensor, offset=0,
                            ap=[[0, P], [1, NUM_FREQS]]))
            w0 = cpool.tile([P, HID], BF16)
            wh = cpool.tile([P, NL - 1, 2, 2, P], BF16)
            wo = cpool.tile([P, 2, DOUT], BF16)
            wos = cpool.tile([P, 2, DOUT], BF16)
            bh = cpool.tile([P, NL - 1, 2], F32)
            bo = cpool.tile([DOUT, 1], F32)
            cb = cpool.tile([P, 2], F32)
            nc.vector.memset(cb[:, 0:1], PI)
            nc.vector.memset(cb[:, 1:2], HALF_PI)

            def load_weights():
                nc.scalar.dma_start(out=w0, in_=w0_d)
                nc.scalar.dma_start(out=wh, in_=wh_d)
                nc.scalar.dma_start(out=wo, in_=wo_d)
                nc.scalar.dma_start(out=wos, in_=wos_d)
                nc.scalar.dma_start(out=bh, in_=bh_d)
                nc.scalar.dma_start(out=bo, in_=bo_d)

            st = {}

            def front(ti, ramp=False):
                """Row-major front-end for one tile: normalize + encode.
                Heavy mul stages go on DVE during the ramp (it's idle)."""
                t, cg0, c = tiles[ti]
                ve = nc.vector if ramp else nc.gpsimd
                x_t = fpool.tile([P, C, 5], F32, tag="x_t", bufs=4)
                nc.gpsimd.dma_start(
                    out=x_t[:, 0:c, :],
                    in_=bass.AP(tensor=xd_d.tensor, offset=cg0 * 5,
                                ap=[[CG * 5, P], [5, c], [1, 5]]))
                d_t = x_t
                rc = fpool.tile([P, C], F32, tag="rc")
                nc.vector.reciprocal(rc[:, 0:c], x_t[:, 0:c, 3])
                xn = fpool.tile([P, C, 4], F32, tag="xn")
                nc.vector.tensor_mul(
                    xn[:, 0:c], x_t[:, 0:c, 0:4],
                    rc[:, 0:c, None].to_broadcast((P, c, 4)))
                nc.gpsimd.tensor_copy(xn[:, 0:c, 3], x_t[:, 0:c, 3])
                # angles in turns: t20[p,c,j,i] = x'_j * 2^(i-1) (exact)
                t20 = fpool.tile([P, C, 4, NUM_FREQS], F32, tag="t20")
                ve.tensor_tensor(
                    t20[:, 0:c],
                    xn[:, 0:c, :, None].to_broadcast((P, c, 4, NUM_FREQS)),
                    fr[:, None, None, :].to_broadcast((P, c, 4, NUM_FREQS)),
                    ALU.mult)
                t20f = t20.rearrange("p c j i -> p c (j i)")
                # m = t - round(t) in [-1/2,1/2] (magic-number round);
                # w = |m|;  sin(2*pi*t) = Sin(2*pi*m);
                # cos(2*pi*t) = cos(2*pi*w) = Sin(pi/2 - 2*pi*w)
                kt = fpool.tile([P, C, 40], F32, tag="kt")
                nc.vector.tensor_scalar(kt[:, 0:c], t20f[:, 0:c],
                                        MAGIC_C, MAGIC_C,
                                        ALU.add, ALU.subtract)
                m20 = fpool.tile([P, C, 40], F16, tag="m20")
                nc.vector.tensor_tensor(m20[:, 0:c], t20f[:, 0:c],
                                        kt[:, 0:c], ALU.subtract)
                w20 = fpool.tile([P, C, 40], F16, tag="w20")
                nc.vector.tensor_scalar(w20.bitcast(U16)[:, 0:c],
                                        m20.bitcast(U16)[:, 0:c],
                                        0x7FFF, None, ALU.bitwise_and)
                # xe rows: [0:4]=x', [4:44]=sin, [44:84]=cos, [84]=1,
                # [85:128]=junk (w0 rows 85:128 are zero)
                xe_r = fpool.tile([P, C, P], BF16, tag="xe_r")
                nc.gpsimd.tensor_copy(xe_r[:, 0:c, 0:4], xn[:, 0:c])
                nc.gpsimd.memset(xe_r[:, 0:c, 84:85], 1.0)
                nc.scalar.activation(xe_r[:, 0:c, 4:44], m20[:, 0:c],
                                     ACTF.Sin, bias=0.0, scale=TWO_PI)
                nc.scalar.activation(xe_r[:, 0:c, 44:84], w20[:, 0:c],
                                     ACTF.Sin, bias=cb[:, 1:2], scale=-TWO_PI)
                # flip to feature-major via DMA XBAR transpose
                xe = apool.tile([P, R], BF16, tag="xe", bufs=4)
                nc.sync.dma_start(
                    out=xe[:, 0:c * P].rearrange("q (c p) -> q c p", c=c),
                    in_=xe_r[:, 0:c, :], transpose=True)
                st[ti] = {"xe": xe, "d": d_t}

            def l0_mm(ti):
                t, cg0, c = tiles[ti]
                xe = st[ti]["xe"]
                za = zpool.tile([P, R], F32, tag="z")
                zb = zpool.tile([P, R], F32, tag="z")
                for half, z in ((0, za), (1, zb)):
                    for c2 in range(c // 4):
                        nc.tensor.matmul(
                            z[:, c2 * 512:(c2 + 1) * 512],
                            w0[0:85, half * P:(half + 1) * P],
                            xe[0:85, c2 * 512:(c2 + 1) * 512],
                            start=True, stop=True)
                st[ti]["z"] = (za, zb)

            def l0_relu(ti):
                t, cg0, c = tiles[ti]
                za, zb = st[ti].pop("z")
                h = apool.tile([P, 2, R], BF16, tag="h", bufs=6)
                w = c * P
                nc.scalar.activation(h[:, 0, 0:w], za[:, 0:w], ACTF.Relu,
                                     bias=0.0, scale=1.0)
                nc.scalar.activation(h[:, 1, 0:w], zb[:, 0:w], ACTF.Relu,
                                     bias=0.0, scale=1.0)
                st[ti]["h"] = h

            def layer_mm(ti, k):
                t, cg0, c = tiles[ti]
                h = st[ti]["h"]
                za = zpool.tile([P, R], F32, tag="z")
                zb = zpool.tile([P, R], F32, tag="z")
                for half, z in ((0, za), (1, zb)):
                    for kin in range(2):
                        for c2 in range(c // 4):
                            nc.tensor.matmul(
                                z[:, c2 * 512:(c2 + 1) * 512],
                                wh[:, k, kin, half, :],
                                h[:, kin, c2 * 512:(c2 + 1) * 512],
                                start=(kin == 0), stop=(kin == 1))
                st[ti]["zk"] = (za, zb)

            def layer_post(ti, k):
                # tt = relu(zk + bh); k<2: h' = s_k*tt + h (TS 4x + TT 2x);
                # k==2: keep tt3 (residual folded into prescaled wos)
                t, cg0, c = tiles[ti]
                za, zb = st[ti].pop("zk")
                h = st[ti]["h"] if k == 2 else st[ti].pop("h")
                w = c * P
                tt = apool.tile([P, 2, R], BF16, tag="tt", bufs=3)
                nc.scalar.activation(tt[:, 0, 0:w], za[:, 0:w], ACTF.Relu,
                                     bias=bh[:, k, 0:1], scale=1.0)
                nc.vector.tensor_scalar(tt[:, 1, 0:w], zb[:, 0:w],
                                        bh[:, k, 1:2], 0.0,
                                        ALU.add, ALU.max)
                if k == 2:
                    st[ti]["t3"] = tt
                    return
                tmp = apool.tile([P, 2, R], BF16, tag="tmp", bufs=3)
                nc.vector.tensor_scalar(tmp[:, :, 0:w], tt[:, :, 0:w],
                                        scl[:, k:k + 1], None, ALU.mult)
                h_new = apool.tile([P, 2, R], BF16, tag="h", bufs=6)
                if k == 0 or c < 8:
                    nc.vector.tensor_tensor(h_new[:, :, 0:w], tmp[:, :, 0:w],
                                            h[:, :, 0:w], ALU.add)
                else:
                    hw = w // 2
                    nc.vector.tensor_tensor(h_new[:, :, 0:hw],
                                            tmp[:, :, 0:hw],
                                            h[:, :, 0:hw], ALU.add)
                    nc.gpsimd.tensor_tensor(h_new[:, :, hw:w],
                                            tmp[:, :, hw:w],
                                            h[:, :, hw:w], ALU.add)
                st[ti]["h"] = h_new

            def out_mm_h2(ti):
                # out-layer h2 part fills the PE while the k2 relus run
                t, cg0, c = tiles[ti]
                h = st[ti]["h"]
                ocs = []
                for c2 in range(c // 4):
                    o_c = opool.tile([DOUT, 512], F32, tag="o")
                    for kin in range(2):
                        nc.tensor.matmul(
                            o_c, wo[:, kin, :],
                            h[:, kin, c2 * 512:(c2 + 1) * 512],
                            start=(kin == 0), stop=False)
                    ocs.append(o_c)
                st[ti]["o"] = ocs

            def out_mm_t3(ti):
                t, cg0, c = tiles[ti]
                tt = st[ti].pop("t3")
                st[ti].pop("h")
                for c2, o_c in enumerate(st[ti]["o"]):
                    for kin in range(2):
                        nc.tensor.matmul(
                            o_c, wos[:, kin, :],
                            tt[:, kin, c2 * 512:(c2 + 1) * 512],
                            start=False, stop=(kin == 1))

            def epilogue(ti):
                t, cg0, c = tiles[ti]
                ocs = st[ti].pop("o")
                d_t = st[ti].pop("d")
                rid = fpool.tile([P, C], F32, tag="rid")
                nc.vector.reciprocal(rid[:, 0:c], d_t[:, 0:c, 4])
                for c2, o_c in enumerate(ocs):
                    oT = fpool.tile([DOUT, 512], BF16, tag="oT", bufs=3)
                    nc.scalar.activation(oT, o_c, ACTF.Identity,
                                         bias=bo, scale=1.0)
                    o_r = fpool.tile([P, 4, DOUT], BF16, tag="o_r", bufs=3)
                    nc.sync.dma_start(out=o_r, in_=oT, transpose=True)
                    o_f = fpool.tile([P, 4, DOUT], F32, tag="o_f", bufs=3)
                    nc.gpsimd.tensor_mul(
                        o_f, o_r,
                        rid[:, c2 * 4:(c2 + 1) * 4, None]
                        .to_broadcast((P, 4, DOUT)))
                    nc.gpsimd.dma_start(
                        out=bass.AP(tensor=out_d.tensor,
                                    offset=(cg0 + c2 * 4) * DOUT,
                                    ap=[[CG * DOUT, P], [DOUT, 4], [1, DOUT]]),
                        in_=o_f)

            # ---- schedule: groups of 3 (post-chain latency < PE window) ----
            groups = [[0, 1, 2], [3, 4, 5], [6, 7, 8]]
            front(0, ramp=True)
            load_weights()
            front(1, ramp=True)
            front(2, ramp=True)
            l0_mm(0)
            l0_mm(1)
            l0_mm(2)
            for gi, group in enumerate(groups):
                for ti in group:
                    l0_relu(ti)
                nxt = groups[gi + 1] if gi + 1 < len(groups) else []
                if nxt:
                    front(nxt[0])
                for k in range(NL - 1):
                    for ti in group:
                        layer_mm(ti, k)
                    if k == 0 and nxt:
                        for ti2 in nxt[1:]:
                            front(ti2)
                    if k == 2:
                        # out-layer h2 part fills the PE gap while the k2
                        # relu chain runs on ACT/DVE
                        for ti in group:
                            out_mm_h2(ti)
                    for ti in group:
                        layer_post(ti, k)
                for ti in group:
                    out_mm_t3(ti)
                for ti in nxt:
                    l0_mm(ti)
                for ti in group:
                    epilogue(ti)

    nc.compile()
    return nc


def _get_program():
    if "nc" not in _compiled:
        _compiled["nc"] = _build_program()
    return _compiled["nc"]


def _xe_perm():
    """perm[slot] = reference xe row for device slot order
    (slots: 0..3 = x', 4 + j*10 + i = sin_{j,i}, 44 + j*10 + i = cos)."""
    perm = np.zeros(84, np.int64)
    perm[0:4] = np.arange(4)
    for s in range(2):
        for j in range(4):
            for i in range(NUM_FREQS):
                perm[4 + s * 40 + j * 10 + i] = 4 + i * 8 + j * 2 + s
    return perm


def _prep_weights(e, W0, b0, Wh, bh, scal, Wout, bout):
    """Host-side layout transforms (permutation / reshape / cast only)."""
    bf = ml_dtypes.bfloat16
    w0 = np.zeros((128, HID), np.float32)
    w0[0:84] = W0[e][_xe_perm()]
    w0[84] = b0[e]
    w0 = w0.astype(bf)
    wh = np.ascontiguousarray(
        Wh[e].reshape(NL - 1, 2, 128, 2, 128)
        .transpose(2, 0, 1, 3, 4)).astype(bf)            # [128,3,2,2,128]
    wo = np.ascontiguousarray(
        Wout[e].reshape(2, 128, DOUT).transpose(1, 0, 2)).astype(bf)
    wos = np.ascontiguousarray(
        (scal[e, 2] * Wout[e]).reshape(2, 128, DOUT)
        .transpose(1, 0, 2)).astype(bf)
    bhr = np.ascontiguousarray(
        bh[e].reshape(NL - 1, 2, 128).transpose(2, 0, 1))  # [128,3,2]
    bor = np.ascontiguousarray(bout[e].reshape(DOUT, 1))
    sc3 = np.ascontiguousarray(scal[e])
    fr10 = (2.0 ** (np.arange(NUM_FREQS, dtype=np.float32) - 1.0)).astype(
        np.float32)
    return dict(w0=w0, wh=wh, wo=wo, wos=wos, bhr=bhr, bor=bor,
                scal3=sc3, fr10=fr10)


def kernel(x, in_dim, layer_id, W0, b0, Wh, bh, scal, Wout, bout):
    from concourse.bass_utils import run_bass_kernel_spmd

    x = np.asarray(x, np.float32)
    in_dim = np.asarray(in_dim, np.float32)
    layer_id = np.asarray(layer_id)
    W0 = np.asarray(W0, np.float32)
    b0 = np.asarray(b0, np.float32)
    Wh = np.asarray(Wh, np.float32)
    bh = np.asarray(bh, np.float32)
    scal = np.asarray(scal, np.float32)
    Wout = np.asarray(Wout, np.float32)
    bout = np.asarray(bout, np.float32)

    # ---- dispatch: expert e -> cores 2e, 2e+1; pad to CAP per core ----
    PADIDX = N
    x_aug = np.vstack([x, np.ones((1, 4), np.float32)])
    d_aug = np.concatenate([in_dim, np.ones(1, np.float32)])
    perms = np.full((NCORE, CAP), PADIDX, np.int64)
    overflow = []
    for e in range(E):
        idx = np.flatnonzero(layer_id == e)
        if len(idx) > 2 * CAP:
            overflow.append(idx[2 * CAP:])
            idx = idx[:2 * CAP]
        nh = min((len(idx) + 1) // 2, CAP)
        perms[2 * e, :nh] = idx[:nh]
        perms[2 * e + 1, :len(idx) - nh] = idx[nh:]

    in_maps = []
    for c in range(NCORE):
        m = _prep_weights(c // 2, W0, b0, Wh, bh, scal, Wout, bout)
        p = perms[c]
        # chunk-major: [...][cg, p] -> [p, cg]; x and in_dim packed
        xd = np.concatenate([x_aug[p], d_aug[p][:, None]], axis=1)
        m["xd_cm"] = np.ascontiguousarray(
            xd.reshape(CG, 128, 5).transpose(1, 0, 2))
        in_maps.append(m)

    nc = _get_program()
    res = run_bass_kernel_spmd(nc, in_maps, core_ids=list(range(NCORE)),
                               **RUN_KWARGS)
    LAST_RESULT.clear()
    LAST_RESULT.append(res)

    out = np.zeros((N + 1, DOUT), np.float32)
    for c in range(NCORE):
        r = np.asarray(res.results[c]["out_cm"], np.float32)
        out[perms[c]] = r.transpose(1, 0, 2).reshape(CAP, DOUT)

    # pathological overflow fallback (never hit for the benchmark input)
    if overflow:
        ov = np.concatenate(overflow)
        out[ov] = _numpy_ref(x[ov], in_dim[ov], layer_id[ov], W0, b0, Wh, bh,
                             scal, Wout, bout)
    return out[:N]


def _numpy_ref(x, in_dim, layer_id, W0, b0, Wh, bh, scal, Wout, bout):
    x = np.concatenate([x[:, :3] / x[:, 3:4], x[:, 3:]], axis=1)
    freqs = (2.0 ** np.arange(NUM_FREQS, dtype=np.float32)) * np.float32(np.pi)
    ang = x[:, None, :] * freqs[None, :, None]
    sc = np.stack([np.sin(ang), np.cos(ang)], axis=-1)
    xe = np.concatenate([x, sc.reshape(x.shape[0], -1)], axis=1)
    out = np.zeros((x.shape[0], DOUT), np.float32)
    for e in range(E):
        m = layer_id == e
        if not m.any():
            continue
        h = np.maximum(xe[m] @ W0[e] + b0[e], 0.0)
        for k in range(NL - 1):
            h = scal[e, k] * np.maximum(h @ Wh[e, k] + bh[e, k], 0.0) + h
        out[m] = h @ Wout[e] + bout[e]
    return out / in_dim[:, None]


# revision 22
# speedup vs baseline: 1.1260x; 1.1260x over previous
"""Trainium2 Bass kernel for nn_NeRF_MLP_Compose (MoE-routed NeRF MLP).

Strategy (v3):
  - Host-side MoE dispatch: expert e's rows split across cores 2e, 2e+1;
    each core runs ONE expert dense over CAP=8704 rows and holds only that
    expert's weights (bf16).
  - Chunk-major DRAM layouts ([128, 68, ...]) so every x/d/out DMA is a
    contiguous 2D transfer (no strided descriptor storms).
  - Front-end (row-major) cut to ~5 ALU ops via a mod-1 range reduction:
      t = x' * 2^(i-1)  (turns);  u = t mod 1;  w = |u - 1/2|
      sin(2*pi*t) = Sin(pi - 2*pi*u);  cos(2*pi*t) = Sin(2*pi*w - pi/2)
    u/w are fp16 (fast DVE modes); one Sin-table ACT per half.
  - R=1024-row tiles (8 full + 1 half): z tiles are [128, 1024] fp32
    2-bank PSUM tiles, halving op/semaphore counts vs 512-row tiles.
  - MLP feature-major bf16: l0 K=96 (ones-row bias), hidden K=256 in
    2x128 chunks, out M=64.  Residual h' = s*t + h as two DVE ops
    (tensor_scalar 4x + tensor_tensor 2x).
  - Relus split ACT(5)/GPSIMD(3); transposes+input DMA on sync queue;
    weights on scalar queue; output stores on gpsimd pseudo-DMA.
"""
import sys
for _p in ("/opt/trn_rl_repo", "/root/.axon_site/_ro/trn_rl_repo"):
    if _p not in sys.path:
        sys.path.insert(0, _p)

import numpy as np
import ml_dtypes

N = 65536
E = 4            # experts
NCORE = 8
CAP = 8704       # rows per core
CG = 68          # 128-row chunks per core
NT = 8           # full 1024-row tiles; +1 half tile
C = 8            # 128-row chunks per full tile
R = 1024         # rows per full tile
NUM_FREQS = 10
HID = 256
DOUT = 64
NL = 4
PI = float(np.float32(np.pi))
TWO_PI = float(np.float32(2 * np.pi))
HALF_PI = float(np.float32(0.5 * np.pi))
MAGIC_C = float(np.float32(1.5 * 2 ** 23))

_compiled = {}
RUN_KWARGS = {}    # test.py may set e.g. {"trace": True}
LAST_RESULT = []   # test.py reads the BassKernelResults appended here


def _build_program():
    import concourse.bass as bass
    from concourse import bacc
    import concourse.mybir as mybir
    import concourse.tile as tile

    F32 = mybir.dt.float32
    F16 = mybir.dt.float16
    U16 = mybir.dt.uint16
    BF16 = mybir.dt.bfloat16
    P = 128
    ALU = mybir.AluOpType
    ACTF = mybir.ActivationFunctionType

    nc = bacc.Bacc("TRN2", target_bir_lowering=False, debug=False)

    # ---- DRAM I/O (per core; one expert's weights) ----
    xd_d = nc.dram_tensor("xd_cm", [P, CG, 5], F32, kind="ExternalInput").ap()
    fr_d = nc.dram_tensor("fr10", [NUM_FREQS], F32, kind="ExternalInput").ap()
    sc_d = nc.dram_tensor("scal3", [NL - 1], F32, kind="ExternalInput").ap()
    w0_d = nc.dram_tensor("w0", [P, HID], BF16, kind="ExternalInput").ap()
    wh_d = nc.dram_tensor("wh", [P, NL - 1, 2, 2, P], BF16,
                          kind="ExternalInput").ap()
    wo_d = nc.dram_tensor("wo", [P, 2, DOUT], BF16, kind="ExternalInput").ap()
    wos_d = nc.dram_tensor("wos", [P, 2, DOUT], BF16, kind="ExternalInput").ap()
    bh_d = nc.dram_tensor("bhr", [P, NL - 1, 2], F32, kind="ExternalInput").ap()
    bo_d = nc.dram_tensor("bor", [DOUT, 1], F32, kind="ExternalInput").ap()
    out_d = nc.dram_tensor("out_cm", [P, CG, DOUT], F32,
                           kind="ExternalOutput").ap()

    # tiles: 8 full (C=8 chunks) + 1 half (4 chunks)
    tiles = [(t, 8 * t, 8) for t in range(NT)] + [(NT, 64, 4)]

    with tile.TileContext(nc) as tc:
        with tc.tile_pool(name="const", bufs=1) as cpool, \
             tc.tile_pool(name="fr", bufs=4) as fpool, \
             tc.tile_pool(name="act", bufs=3) as apool, \
             tc.tile_pool(name="psz", bufs=3, space="PSUM") as zpool, \
             tc.tile_pool(name="pso", bufs=1, space="PSUM") as opool:

            # ---- front-end constants first ----
            scl = cpool.tile([P, NL - 1], F32)
            nc.sync.dma_start(
                out=scl,
                in_=bass.AP(tensor=sc_d.tensor, offset=0,
                            ap=[[0, P], [1, NL - 1]]))
            fr = cpool.tile([P, NUM_FREQS], F32)
            nc.sync.dma_start(
                out=fr,
                in_=bass.AP(tensor=fr_d.tensor, offset=0,
                            ap=[[0, P], [1, NUM_FREQS]]))
            w0 = cpool.tile([P, HID], BF16)
            wh = cpool.tile([P, NL - 1, 2, 2, P], BF16)
            wo = cpool.tile([P, 2, DOUT], BF16)
            wos = cpool.tile([P, 2, DOUT], BF16)
            bh = cpool.tile([P, NL - 1, 2], F32)
            bo = cpool.tile([DOUT, 1], F32)
            cb = cpool.tile([P, 2], F32)
            nc.vector.memset(cb[:, 0:1], PI)
            nc.vector.memset(cb[:, 1:2], HALF_PI)

            def load_weights():
                nc.scalar.dma_start(out=w0, in_=w0_d)
                nc.scalar.dma_start(out=wh, in_=wh_d)
                nc.scalar.dma_start(out=wo, in_=wo_d)
                nc.scalar.dma_start(out=wos, in_=wos_d)
                nc.scalar.dma_start(out=bh, in_=bh_d)
                nc.scalar.dma_start(out=bo, in_=bo_d)

            st = {}
            xts = {}

            def stage_x(ti):
                t, cg0, c = tiles[ti]
                x_t = fpool.tile([P, C, 5], F32, tag="x_t", bufs=9)
                nc.scalar.dma_start(
                    out=x_t[:, 0:c, :],
                    in_=bass.AP(tensor=xd_d.tensor, offset=cg0 * 5,
                                ap=[[CG * 5, P], [5, c], [1, 5]]))
                xts[ti] = x_t

            def front_pre(ti):
                """Range-reduction chain (DVE/GPSIMD): x' -> m20/w20."""
                t, cg0, c = tiles[ti]
                x_t = xts[ti]
                rc = fpool.tile([P, C], F32, tag="rc")
                nc.vector.reciprocal(rc[:, 0:c], x_t[:, 0:c, 3])
                xn = fpool.tile([P, C, 4], F32, tag="xn")
                nc.vector.tensor_mul(
                    xn[:, 0:c], x_t[:, 0:c, 0:4],
                    rc[:, 0:c, None].to_broadcast((P, c, 4)))
                nc.gpsimd.tensor_copy(xn[:, 0:c, 3], x_t[:, 0:c, 3])
                # angles in turns: t20[p,c,j,i] = x'_j * 2^(i-1) (exact)
                t20 = fpool.tile([P, C, 4, NUM_FREQS], F32, tag="t20")
                nc.gpsimd.tensor_tensor(
                    t20[:, 0:c],
                    xn[:, 0:c, :, None].to_broadcast((P, c, 4, NUM_FREQS)),
                    fr[:, None, None, :].to_broadcast((P, c, 4, NUM_FREQS)),
                    ALU.mult)
                t20f = t20.rearrange("p c j i -> p c (j i)")
                # m = t - round(t) in [-1/2,1/2] (magic-number round);
                # w = |m|;  sin(2*pi*t) = Sin(2*pi*m);
                # cos(2*pi*t) = cos(2*pi*w) = Sin(pi/2 - 2*pi*w)
                kt = fpool.tile([P, C, 40], F32, tag="kt")
                nc.vector.tensor_scalar(kt[:, 0:c], t20f[:, 0:c],
                                        MAGIC_C, MAGIC_C,
                                        ALU.add, ALU.subtract)
                m20 = fpool.tile([P, C, 40], F16, tag="m20")
                nc.gpsimd.tensor_tensor(m20[:, 0:c], t20f[:, 0:c],
                                        kt[:, 0:c], ALU.subtract)
                w20 = fpool.tile([P, C, 40], F16, tag="w20")
                nc.vector.tensor_scalar(w20.bitcast(U16)[:, 0:c],
                                        m20.bitcast(U16)[:, 0:c],
                                        0x7FFF, None, ALU.bitwise_and)
                st[ti] = {"d": x_t, "xn": xn, "m20": m20, "w20": w20}

            def front_post(ti):
                """Sin/cos (ACT) + transpose to feature-major."""
                t, cg0, c = tiles[ti]
                xn = st[ti].pop("xn")
                m20 = st[ti].pop("m20")
                w20 = st[ti].pop("w20")
                # xe rows: [0:4]=x', [4:44]=sin, [44:84]=cos, [84]=1,
                # [85:128]=junk (never read: l0 matmuls use K=85)
                xe_r = fpool.tile([P, C, P], BF16, tag="xe_r")
                nc.gpsimd.tensor_copy(xe_r[:, 0:c, 0:4], xn[:, 0:c])
                nc.gpsimd.memset(xe_r[:, 0:c, 84:85], 1.0)
                nc.scalar.activation(xe_r[:, 0:c, 4:44], m20[:, 0:c],
                                     ACTF.Sin, bias=0.0, scale=TWO_PI)
                nc.scalar.activation(xe_r[:, 0:c, 44:84], w20[:, 0:c],
                                     ACTF.Sin, bias=cb[:, 1:2], scale=-TWO_PI)
                # flip to feature-major via DMA XBAR transpose
                xe = apool.tile([P, R], BF16, tag="xe", bufs=4)
                nc.sync.dma_start(
                    out=xe[:, 0:c * P].rearrange("q (c p) -> q c p", c=c),
                    in_=xe_r[:, 0:c, :], transpose=True)
                st[ti]["xe"] = xe

            def front(ti, ramp=False):
                front_pre(ti)
                front_post(ti)

            def l0_mm(ti):
                t, cg0, c = tiles[ti]
                xe = st[ti]["xe"]
                za = zpool.tile([P, R], F32, tag="z")
                zb = zpool.tile([P, R], F32, tag="z")
                for half, z in ((0, za), (1, zb)):
                    for c2 in range(c // 4):
                        nc.tensor.matmul(
                            z[:, c2 * 512:(c2 + 1) * 512],
                            w0[0:85, half * P:(half + 1) * P],
                            xe[0:85, c2 * 512:(c2 + 1) * 512],
                            start=True, stop=True)
                st[ti]["z"] = (za, zb)

            def l0_relu(ti):
                t, cg0, c = tiles[ti]
                za, zb = st[ti].pop("z")
                h = apool.tile([P, 2, R], BF16, tag="h", bufs=6)
                w = c * P
                nc.scalar.activation(h[:, 0, 0:w], za[:, 0:w], ACTF.Relu,
                                     bias=0.0, scale=1.0)
                nc.scalar.activation(h[:, 1, 0:w], zb[:, 0:w], ACTF.Relu,
                                     bias=0.0, scale=1.0)
                st[ti]["h"] = h

            def layer_mm(ti, k):
                t, cg0, c = tiles[ti]
                h = st[ti]["h"]
                za = zpool.tile([P, R], F32, tag="z")
                zb = zpool.tile([P, R], F32, tag="z")
                for half, z in ((0, za), (1, zb)):
                    for kin in range(2):
                        for c2 in range(c // 4):
                            nc.tensor.matmul(
                                z[:, c2 * 512:(c2 + 1) * 512],
                                wh[:, k, kin, half, :],
                                h[:, kin, c2 * 512:(c2 + 1) * 512],
                                start=(kin == 0), stop=(kin == 1))
                st[ti]["zk"] = (za, zb)

            def layer_post(ti, k):
                # tt = relu(zk + bh); k<2: h' = s_k*tt + h (TS 4x + TT 2x);
                # k==2: keep tt3 (residual folded into prescaled wos)
                t, cg0, c = tiles[ti]
                za, zb = st[ti].pop("zk")
                h = st[ti]["h"] if k == 2 else st[ti].pop("h")
                w = c * P
                tt = apool.tile([P, 2, R], BF16, tag="tt", bufs=3)
                nc.scalar.activation(tt[:, 0, 0:w], za[:, 0:w], ACTF.Relu,
                                     bias=bh[:, k, 0:1], scale=1.0)
                nc.vector.tensor_scalar(tt[:, 1, 0:w], zb[:, 0:w],
                                        bh[:, k, 1:2], 0.0,
                                        ALU.add, ALU.max)
                if k == 2:
                    st[ti]["t3"] = tt
                    return
                tmp = apool.tile([P, 2, R], BF16, tag="tmp", bufs=3)
                nc.vector.tensor_scalar(tmp[:, :, 0:w], tt[:, :, 0:w],
                                        scl[:, k:k + 1], None, ALU.mult)
                h_new = apool.tile([P, 2, R], BF16, tag="h", bufs=6)
                if c < 8:
                    nc.vector.tensor_tensor(h_new[:, :, 0:w], tmp[:, :, 0:w],
                                            h[:, :, 0:w], ALU.add)
                else:
                    hw = w // 2
                    nc.vector.tensor_tensor(h_new[:, :, 0:hw],
                                            tmp[:, :, 0:hw],
                                            h[:, :, 0:hw], ALU.add)
                    nc.gpsimd.tensor_tensor(h_new[:, :, hw:w],
                                            tmp[:, :, hw:w],
                                            h[:, :, hw:w], ALU.add)
                st[ti]["h"] = h_new

            def out_mm_h2(ti):
                # out-layer h2 part fills the PE while the k2 relus run
                t, cg0, c = tiles[ti]
                h = st[ti]["h"]
                ocs = []
                for c2 in range(c // 4):
                    o_c = opool.tile([DOUT, 512], F32, tag="o")
                    for kin in range(2):
                        nc.tensor.matmul(
                            o_c, wo[:, kin, :],
                            h[:, kin, c2 * 512:(c2 + 1) * 512],
                            start=(kin == 0), stop=False)
                    ocs.append(o_c)
                st[ti]["o"] = ocs

            def out_mm_t3(ti):
                t, cg0, c = tiles[ti]
                tt = st[ti].pop("t3")
                st[ti].pop("h")
                for c2, o_c in enumerate(st[ti]["o"]):
                    for kin in range(2):
                        nc.tensor.matmul(
                            o_c, wos[:, kin, :],
                            tt[:, kin, c2 * 512:(c2 + 1) * 512],
                            start=False, stop=(kin == 1))

            def epilogue(ti):
                t, cg0, c = tiles[ti]
                ocs = st[ti].pop("o")
                d_t = st[ti].pop("d")
                w = c * P
                rid = fpool.tile([P, C], F32, tag="rid")
                nc.vector.reciprocal(rid[:, 0:c], d_t[:, 0:c, 4])
                oT = fpool.tile([DOUT, R], BF16, tag="oT", bufs=2)
                for c2, o_c in enumerate(ocs):
                    nc.vector.tensor_scalar(oT[:, c2 * 512:(c2 + 1) * 512],
                                            o_c, bo[:, 0:1], None, ALU.add)
                o_r = fpool.tile([P, C, DOUT], BF16, tag="o_r", bufs=2)
                nc.sync.dma_start(out=o_r[:, 0:c, :], in_=oT[:, 0:w],
                                  transpose=True)
                o_f = fpool.tile([P, C, DOUT], F32, tag="o_f", bufs=2)
                nc.gpsimd.tensor_mul(
                    o_f[:, 0:c], o_r[:, 0:c],
                    rid[:, 0:c, None].to_broadcast((P, c, DOUT)))
                nc.sync.dma_start(
                    out=bass.AP(tensor=out_d.tensor, offset=cg0 * DOUT,
                                ap=[[CG * DOUT, P], [DOUT, c], [1, DOUT]]),
                    in_=o_f[:, 0:c])

            # ---- schedule: groups of 3 (post-chain latency < PE window);
            # all x DMAs staged up-front; next group's front range-reduction
            # right after this group's l0 relus, sin/cos+transpose at k==1
            groups = [[0, 1, 2], [3, 4, 5], [6, 7, 8]]
            nc.scalar.dma_start(out=w0, in_=w0_d)
            for ti in range(9):
                stage_x(ti)
            nc.scalar.dma_start(out=wh, in_=wh_d)
            nc.scalar.dma_start(out=wo, in_=wo_d)
            nc.scalar.dma_start(out=wos, in_=wos_d)
            nc.scalar.dma_start(out=bh, in_=bh_d)
            nc.scalar.dma_start(out=bo, in_=bo_d)
            for ti in groups[0]:
                front(ti, ramp=True)
            for ti in groups[0]:
                l0_mm(ti)
            for gi, group in enumerate(groups):
                for ti in group:
                    l0_relu(ti)
                nxt = groups[gi + 1] if gi + 1 < len(groups) else []
                for ti2 in nxt:
                    front_pre(ti2)
                for k in range(NL - 1):
                    for ti in group:
                        layer_mm(ti, k)
                    if k == 1:
                        for ti2 in nxt:
                            front_post(ti2)
                    if k == 2:
                        # out-layer h2 part fills the PE gap while the k2
                        # relu chain runs on ACT/DVE
                        for ti in group:
                            out_mm_h2(ti)
                    for ti in group:
                        layer_post(ti, k)
                for ti in group:
                    out_mm_t3(ti)
                for ti in nxt:
                    l0_mm(ti)
                for ti in group:
                    epilogue(ti)

    nc.compile()
    return nc


def _get_program():
    if "nc" not in _compiled:
        _compiled["nc"] = _build_program()
    return _compiled["nc"]


def _xe_perm():
    """perm[slot] = reference xe row for device slot order
    (slots: 0..3 = x', 4 + j*10 + i = sin_{j,i}, 44 + j*10 + i = cos)."""
    perm = np.zeros(84, np.int64)
    perm[0:4] = np.arange(4)
    for s in range(2):
        for j in range(4):
            for i in range(NUM_FREQS):
                perm[4 + s * 40 + j * 10 + i] = 4 + i * 8 + j * 2 + s
    return perm


def _prep_weights(e, W0, b0, Wh, bh, scal, Wout, bout):
    """Host-side layout transforms (permutation / reshape / cast only)."""
    bf = ml_dtypes.bfloat16
    w0 = np.zeros((128, HID), np.float32)
    w0[0:84] = W0[e][_xe_perm()]
    w0[84] = b0[e]
    w0 = w0.astype(bf)
    wh = np.ascontiguousarray(
        Wh[e].reshape(NL - 1, 2, 128, 2, 128)
        .transpose(2, 0, 1, 3, 4)).astype(bf)            # [128,3,2,2,128]
    wo = np.ascontiguousarray(
        Wout[e].reshape(2, 128, DOUT).transpose(1, 0, 2)).astype(bf)
    wos = np.ascontiguousarray(
        (scal[e, 2] * Wout[e]).reshape(2, 128, DOUT)
        .transpose(1, 0, 2)).astype(bf)
    bhr = np.ascontiguousarray(
        bh[e].reshape(NL - 1, 2, 128).transpose(2, 0, 1))  # [128,3,2]
    bor = np.ascontiguousarray(bout[e].reshape(DOUT, 1))
    sc3 = np.ascontiguousarray(scal[e])
    fr10 = (2.0 ** (np.arange(NUM_FREQS, dtype=np.float32) - 1.0)).astype(
        np.float32)
    return dict(w0=w0, wh=wh, wo=wo, wos=wos, bhr=bhr, bor=bor,
                scal3=sc3, fr10=fr10)


def kernel(x, in_dim, layer_id, W0, b0, Wh, bh, scal, Wout, bout):
    from concourse.bass_utils import run_bass_kernel_spmd

    x = np.asarray(x, np.float32)
    in_dim = np.asarray(in_dim, np.float32)
    layer_id = np.asarray(layer_id)
    W0 = np.asarray(W0, np.float32)
    b0 = np.asarray(b0, np.float32)
    Wh = np.asarray(Wh, np.float32)
    bh = np.asarray(bh, np.float32)
    scal = np.asarray(scal, np.float32)
    Wout = np.asarray(Wout, np.float32)
    bout = np.asarray(bout, np.float32)

    # ---- dispatch: expert e -> cores 2e, 2e+1; pad to CAP per core ----
    PADIDX = N
    x_aug = np.vstack([x, np.ones((1, 4), np.float32)])
    d_aug = np.concatenate([in_dim, np.ones(1, np.float32)])
    perms = np.full((NCORE, CAP), PADIDX, np.int64)
    overflow = []
    for e in range(E):
        idx = np.flatnonzero(layer_id == e)
        if len(idx) > 2 * CAP:
            overflow.append(idx[2 * CAP:])
            idx = idx[:2 * CAP]
        nh = min((len(idx) + 1) // 2, CAP)
        perms[2 * e, :nh] = idx[:nh]
        perms[2 * e + 1, :len(idx) - nh] = idx[nh:]

    in_maps = []
    for c in range(NCORE):
        m = _prep_weights(c // 2, W0, b0, Wh, bh, scal, Wout, bout)
        p = perms[c]
        # chunk-major: [...][cg, p] -> [p, cg]; x and in_dim packed
        xd = np.concatenate([x_aug[p], d_aug[p][:, None]], axis=1)
        m["xd_cm"] = np.ascontiguousarray(
            xd.reshape(CG, 128, 5).transpose(1, 0, 2))
        in_maps.append(m)

    nc = _get_program()
    res = run_bass_kernel_spmd(nc, in_maps, core_ids=list(range(NCORE)),
                               **RUN_KWARGS)
    LAST_RESULT.clear()
    LAST_RESULT.append(res)

    out = np.zeros((N + 1, DOUT), np.float32)
    for c in range(NCORE):
        r = np.asarray(res.results[c]["out_cm"], np.float32)
        out[perms[c]] = r.transpose(1, 0, 2).reshape(CAP, DOUT)

    # pathological overflow fallback (never hit for the benchmark input)
    if overflow:
        ov = np.concatenate(overflow)
        out[ov] = _numpy_ref(x[ov], in_dim[ov], layer_id[ov], W0, b0, Wh, bh,
                             scal, Wout, bout)
    return out[:N]


def _numpy_ref(x, in_dim, layer_id, W0, b0, Wh, bh, scal, Wout, bout):
    x = np.concatenate([x[:, :3] / x[:, 3:4], x[:, 3:]], axis=1)
    freqs = (2.0 ** np.arange(NUM_FREQS, dtype=np.float32)) * np.float32(np.pi)
    ang = x[:, None, :] * freqs[None, :, None]
    sc = np.stack([np.sin(ang), np.cos(ang)], axis=-1)
    xe = np.concatenate([x, sc.reshape(x.shape[0], -1)], axis=1)
    out = np.zeros((x.shape[0], DOUT), np.float32)
    for e in range(E):
        m = layer_id == e
        if not m.any():
            continue
        h = np.maximum(xe[m] @ W0[e] + b0[e], 0.0)
        for k in range(NL - 1):
            h = scal[e, k] * np.maximum(h @ Wh[e, k] + bh[e, k], 0.0) + h
        out[m] = h @ Wout[e] + bout[e]
    return out / in_dim[:, None]


# revision 23
# speedup vs baseline: 1.1722x; 1.0411x over previous
"""Trainium2 Bass kernel for nn_NeRF_MLP_Compose (MoE-routed NeRF MLP).

Strategy (v3):
  - Host-side MoE dispatch: expert e's rows split across cores 2e, 2e+1;
    each core runs ONE expert dense over CAP=8704 rows and holds only that
    expert's weights (bf16).
  - Chunk-major DRAM layouts ([128, 68, ...]) so every x/d/out DMA is a
    contiguous 2D transfer (no strided descriptor storms).
  - Front-end (row-major) cut to ~5 ALU ops via a mod-1 range reduction:
      t = x' * 2^(i-1)  (turns);  u = t mod 1;  w = |u - 1/2|
      sin(2*pi*t) = Sin(pi - 2*pi*u);  cos(2*pi*t) = Sin(2*pi*w - pi/2)
    u/w are fp16 (fast DVE modes); one Sin-table ACT per half.
  - R=1024-row tiles (8 full + 1 half): z tiles are [128, 1024] fp32
    2-bank PSUM tiles, halving op/semaphore counts vs 512-row tiles.
  - MLP feature-major bf16: l0 K=96 (ones-row bias), hidden K=256 in
    2x128 chunks, out M=64.  Residual h' = s*t + h as two DVE ops
    (tensor_scalar 4x + tensor_tensor 2x).
  - Relus split ACT(5)/GPSIMD(3); transposes+input DMA on sync queue;
    weights on scalar queue; output stores on gpsimd pseudo-DMA.
"""
import sys
for _p in ("/opt/trn_rl_repo", "/root/.axon_site/_ro/trn_rl_repo"):
    if _p not in sys.path:
        sys.path.insert(0, _p)

import numpy as np
import ml_dtypes

N = 65536
E = 4            # experts
NCORE = 8
CAP = 8704       # rows per core
CG = 68          # 128-row chunks per core
NT = 8           # full 1024-row tiles; +1 half tile
C = 8            # 128-row chunks per full tile
R = 1024         # rows per full tile
NUM_FREQS = 10
HID = 256
DOUT = 64
NL = 4
PI = float(np.float32(np.pi))
TWO_PI = float(np.float32(2 * np.pi))
HALF_PI = float(np.float32(0.5 * np.pi))
MAGIC_C = float(np.float32(1.5 * 2 ** 23))

_compiled = {}
RUN_KWARGS = {}    # test.py may set e.g. {"trace": True}
LAST_RESULT = []   # test.py reads the BassKernelResults appended here


def _build_program():
    import concourse.bass as bass
    from concourse import bacc
    import concourse.mybir as mybir
    import concourse.tile as tile

    F32 = mybir.dt.float32
    F16 = mybir.dt.float16
    U16 = mybir.dt.uint16
    BF16 = mybir.dt.bfloat16
    P = 128
    ALU = mybir.AluOpType
    ACTF = mybir.ActivationFunctionType

    nc = bacc.Bacc("TRN2", target_bir_lowering=False, debug=False)

    # ---- DRAM I/O (per core; one expert's weights) ----
    xd_d = nc.dram_tensor("xd_cm", [P, CG, 5], F32, kind="ExternalInput").ap()
    fr_d = nc.dram_tensor("fr10", [NUM_FREQS], F32, kind="ExternalInput").ap()
    sc_d = nc.dram_tensor("scal3", [NL - 1], F32, kind="ExternalInput").ap()
    w0_d = nc.dram_tensor("w0", [P, HID], BF16, kind="ExternalInput").ap()
    wh_d = nc.dram_tensor("wh", [P, NL - 1, 2, 2, P], BF16,
                          kind="ExternalInput").ap()
    wo_d = nc.dram_tensor("wo", [P, 2, DOUT], BF16, kind="ExternalInput").ap()
    wos_d = nc.dram_tensor("wos", [P, 2, DOUT], BF16, kind="ExternalInput").ap()
    bh_d = nc.dram_tensor("bhr", [P, NL - 1, 2], F32, kind="ExternalInput").ap()
    bo_d = nc.dram_tensor("bor", [DOUT, 1], F32, kind="ExternalInput").ap()
    out_d = nc.dram_tensor("out_cm", [P, CG, DOUT], F32,
                           kind="ExternalOutput").ap()

    # tiles: 8 full (C=8 chunks) + 1 half (4 chunks)
    tiles = [(t, 8 * t, 8) for t in range(NT)] + [(NT, 64, 4)]

    with tile.TileContext(nc) as tc:
        with tc.tile_pool(name="const", bufs=1) as cpool, \
             tc.tile_pool(name="fr", bufs=4) as fpool, \
             tc.tile_pool(name="act", bufs=3) as apool, \
             tc.tile_pool(name="psz", bufs=3, space="PSUM") as zpool, \
             tc.tile_pool(name="pso", bufs=1, space="PSUM") as opool:

            # ---- front-end constants first ----
            scl = cpool.tile([P, NL - 1], F32)
            nc.sync.dma_start(
                out=scl,
                in_=bass.AP(tensor=sc_d.tensor, offset=0,
                            ap=[[0, P], [1, NL - 1]]))
            fr = cpool.tile([P, NUM_FREQS], F32)
            nc.sync.dma_start(
                out=fr,
                in_=bass.AP(tensor=fr_d.tensor, offset=0,
                            ap=[[0, P], [1, NUM_FREQS]]))
            w0 = cpool.tile([P, HID], BF16)
            wh = cpool.tile([P, NL - 1, 2, 2, P], BF16)
            wo = cpool.tile([P, 2, DOUT], BF16)
            wos = cpool.tile([P, 2, DOUT], BF16)
            bh = cpool.tile([P, NL - 1, 2], F32)
            bo = cpool.tile([DOUT, 1], F32)
            cb = cpool.tile([P, 2], F32)
            nc.vector.memset(cb[:, 0:1], PI)
            nc.vector.memset(cb[:, 1:2], HALF_PI)

            def load_weights():
                nc.scalar.dma_start(out=w0, in_=w0_d)
                nc.scalar.dma_start(out=wh, in_=wh_d)
                nc.scalar.dma_start(out=wo, in_=wo_d)
                nc.scalar.dma_start(out=wos, in_=wos_d)
                nc.scalar.dma_start(out=bh, in_=bh_d)
                nc.scalar.dma_start(out=bo, in_=bo_d)

            st = {}
            xts = {}

            def stage_x(ti):
                t, cg0, c = tiles[ti]
                x_t = fpool.tile([P, C, 5], F32, tag="x_t", bufs=9)
                nc.scalar.dma_start(
                    out=x_t[:, 0:c, :],
                    in_=bass.AP(tensor=xd_d.tensor, offset=cg0 * 5,
                                ap=[[CG * 5, P], [5, c], [1, 5]]))
                xts[ti] = x_t

            def front_pre(ti):
                """Range-reduction chain (DVE/GPSIMD): x' -> m20/w20."""
                t, cg0, c = tiles[ti]
                x_t = xts[ti]
                rc = fpool.tile([P, C], F32, tag="rc")
                nc.vector.reciprocal(rc[:, 0:c], x_t[:, 0:c, 3])
                xn = fpool.tile([P, C, 4], F32, tag="xn")
                nc.vector.tensor_mul(
                    xn[:, 0:c], x_t[:, 0:c, 0:4],
                    rc[:, 0:c, None].to_broadcast((P, c, 4)))
                nc.gpsimd.tensor_copy(xn[:, 0:c, 3], x_t[:, 0:c, 3])
                # angles in turns: t20[p,c,j,i] = x'_j * 2^(i-1) (exact)
                t20 = fpool.tile([P, C, 4, NUM_FREQS], F32, tag="t20")
                nc.gpsimd.tensor_tensor(
                    t20[:, 0:c],
                    xn[:, 0:c, :, None].to_broadcast((P, c, 4, NUM_FREQS)),
                    fr[:, None, None, :].to_broadcast((P, c, 4, NUM_FREQS)),
                    ALU.mult)
                t20f = t20.rearrange("p c j i -> p c (j i)")
                # m = t - round(t) in [-1/2,1/2] (magic-number round);
                # w = |m|;  sin(2*pi*t) = Sin(2*pi*m);
                # cos(2*pi*t) = cos(2*pi*w) = Sin(pi/2 - 2*pi*w)
                kt = fpool.tile([P, C, 40], F32, tag="kt")
                nc.vector.tensor_scalar(kt[:, 0:c], t20f[:, 0:c],
                                        MAGIC_C, MAGIC_C,
                                        ALU.add, ALU.subtract)
                m20 = fpool.tile([P, C, 40], F16, tag="m20")
                nc.gpsimd.tensor_tensor(m20[:, 0:c], t20f[:, 0:c],
                                        kt[:, 0:c], ALU.subtract)
                w20 = fpool.tile([P, C, 40], F16, tag="w20")
                nc.vector.tensor_scalar(w20.bitcast(U16)[:, 0:c],
                                        m20.bitcast(U16)[:, 0:c],
                                        0x7FFF, None, ALU.bitwise_and)
                st[ti] = {"d": x_t, "xn": xn, "m20": m20, "w20": w20}

            def front_post(ti):
                """Sin/cos (ACT) + transpose to feature-major."""
                t, cg0, c = tiles[ti]
                xn = st[ti].pop("xn")
                m20 = st[ti].pop("m20")
                w20 = st[ti].pop("w20")
                # xe rows: [0:4]=x', [4:44]=sin, [44:84]=cos, [84]=1,
                # [85:128]=junk (never read: l0 matmuls use K=85)
                xe_r = fpool.tile([P, C, P], BF16, tag="xe_r")
                nc.gpsimd.tensor_copy(xe_r[:, 0:c, 0:4], xn[:, 0:c])
                nc.gpsimd.memset(xe_r[:, 0:c, 84:85], 1.0)
                nc.scalar.activation(xe_r[:, 0:c, 4:44], m20[:, 0:c],
                                     ACTF.Sin, bias=0.0, scale=TWO_PI)
                nc.scalar.activation(xe_r[:, 0:c, 44:84], w20[:, 0:c],
                                     ACTF.Sin, bias=cb[:, 1:2], scale=-TWO_PI)
                # flip to feature-major via DMA XBAR transpose
                xe = apool.tile([P, R], BF16, tag="xe", bufs=4)
                nc.sync.dma_start(
                    out=xe[:, 0:c * P].rearrange("q (c p) -> q c p", c=c),
                    in_=xe_r[:, 0:c, :], transpose=True)
                st[ti]["xe"] = xe

            def front(ti, ramp=False):
                front_pre(ti)
                front_post(ti)

            def l0_mm(ti):
                t, cg0, c = tiles[ti]
                xe = st[ti]["xe"]
                za = zpool.tile([P, R], F32, tag="z")
                zb = zpool.tile([P, R], F32, tag="z")
                for half, z in ((0, za), (1, zb)):
                    for c2 in range(c // 4):
                        nc.tensor.matmul(
                            z[:, c2 * 512:(c2 + 1) * 512],
                            w0[0:85, half * P:(half + 1) * P],
                            xe[0:85, c2 * 512:(c2 + 1) * 512],
                            start=True, stop=True)
                st[ti]["z"] = (za, zb)

            def l0_relu(ti):
                t, cg0, c = tiles[ti]
                za, zb = st[ti].pop("z")
                h0 = apool.tile([P, R], BF16, tag="h", bufs=12)
                h1 = apool.tile([P, R], BF16, tag="h", bufs=12)
                w = c * P
                nc.scalar.activation(h0[:, 0:w], za[:, 0:w], ACTF.Relu,
                                     bias=0.0, scale=1.0)
                nc.scalar.activation(h1[:, 0:w], zb[:, 0:w], ACTF.Relu,
                                     bias=0.0, scale=1.0)
                st[ti]["h"] = (h0, h1)

            def layer_mm(ti, k):
                t, cg0, c = tiles[ti]
                hk = st[ti]["h"]
                za = zpool.tile([P, R], F32, tag="z")
                zb = zpool.tile([P, R], F32, tag="z")
                for half, z in ((0, za), (1, zb)):
                    for kin in range(2):
                        for c2 in range(c // 4):
                            nc.tensor.matmul(
                                z[:, c2 * 512:(c2 + 1) * 512],
                                wh[:, k, kin, half, :],
                                hk[kin][:, c2 * 512:(c2 + 1) * 512],
                                start=(kin == 0), stop=(kin == 1))
                st[ti]["zk"] = (za, zb)

            def layer_post(ti, k):
                # tt = relu(zk + bh); k<2: h' = s_k*tt + h, per half so the
                # next layer's kin0 matmuls only wait on half 0;
                # k==2: keep tt3 (residual folded into prescaled wos)
                t, cg0, c = tiles[ti]
                za, zb = st[ti].pop("zk")
                hk = st[ti]["h"] if k == 2 else st[ti].pop("h")
                w = c * P
                tt0 = apool.tile([P, R], BF16, tag="tt", bufs=6)
                tt1 = apool.tile([P, R], BF16, tag="tt", bufs=6)
                nc.scalar.activation(tt0[:, 0:w], za[:, 0:w], ACTF.Relu,
                                     bias=bh[:, k, 0:1], scale=1.0)
                nc.vector.tensor_scalar(tt1[:, 0:w], zb[:, 0:w],
                                        bh[:, k, 1:2], 0.0,
                                        ALU.add, ALU.max)
                if k == 2:
                    st[ti]["t3"] = (tt0, tt1)
                    return
                nh = []
                for half, tth in ((0, tt0), (1, tt1)):
                    tmp = apool.tile([P, R], BF16, tag="tmp", bufs=6)
                    nc.vector.tensor_scalar(tmp[:, 0:w], tth[:, 0:w],
                                            scl[:, k:k + 1], None, ALU.mult)
                    h_new = apool.tile([P, R], BF16, tag="h", bufs=12)
                    ve = nc.gpsimd if (half == 1 and c == 8) else nc.vector
                    ve.tensor_tensor(h_new[:, 0:w], tmp[:, 0:w],
                                     hk[half][:, 0:w], ALU.add)
                    nh.append(h_new)
                st[ti]["h"] = tuple(nh)

            def out_mm_h2(ti):
                # out-layer h2 part fills the PE while the k2 relus run
                t, cg0, c = tiles[ti]
                hk = st[ti]["h"]
                ocs = []
                for c2 in range(c // 4):
                    o_c = opool.tile([DOUT, 512], F32, tag="o")
                    for kin in range(2):
                        nc.tensor.matmul(
                            o_c, wo[:, kin, :],
                            hk[kin][:, c2 * 512:(c2 + 1) * 512],
                            start=(kin == 0), stop=False)
                    ocs.append(o_c)
                st[ti]["o"] = ocs

            def out_mm_t3(ti):
                t, cg0, c = tiles[ti]
                tt = st[ti].pop("t3")
                st[ti].pop("h")
                for c2, o_c in enumerate(st[ti]["o"]):
                    for kin in range(2):
                        nc.tensor.matmul(
                            o_c, wos[:, kin, :],
                            tt[kin][:, c2 * 512:(c2 + 1) * 512],
                            start=False, stop=(kin == 1))

            def epilogue(ti):
                t, cg0, c = tiles[ti]
                ocs = st[ti].pop("o")
                d_t = st[ti].pop("d")
                w = c * P
                rid = fpool.tile([P, C], F32, tag="rid")
                nc.vector.reciprocal(rid[:, 0:c], d_t[:, 0:c, 4])
                oT = fpool.tile([DOUT, R], BF16, tag="oT", bufs=2)
                for c2, o_c in enumerate(ocs):
                    nc.vector.tensor_scalar(oT[:, c2 * 512:(c2 + 1) * 512],
                                            o_c, bo[:, 0:1], None, ALU.add)
                o_r = fpool.tile([P, C, DOUT], BF16, tag="o_r", bufs=2)
                nc.sync.dma_start(out=o_r[:, 0:c, :], in_=oT[:, 0:w],
                                  transpose=True)
                o_f = fpool.tile([P, C, DOUT], F32, tag="o_f", bufs=2)
                nc.gpsimd.tensor_mul(
                    o_f[:, 0:c], o_r[:, 0:c],
                    rid[:, 0:c, None].to_broadcast((P, c, DOUT)))
                nc.sync.dma_start(
                    out=bass.AP(tensor=out_d.tensor, offset=cg0 * DOUT,
                                ap=[[CG * DOUT, P], [DOUT, c], [1, DOUT]]),
                    in_=o_f[:, 0:c])

            # ---- schedule: groups of 3 (post-chain latency < PE window);
            # all x DMAs staged up-front; next group's front range-reduction
            # right after this group's l0 relus, sin/cos+transpose at k==1
            groups = [[0, 1, 2], [3, 4, 5], [6, 7, 8]]
            nc.scalar.dma_start(out=w0, in_=w0_d)
            for ti in range(9):
                stage_x(ti)
            nc.scalar.dma_start(out=wh, in_=wh_d)
            nc.scalar.dma_start(out=wo, in_=wo_d)
            nc.scalar.dma_start(out=wos, in_=wos_d)
            nc.scalar.dma_start(out=bh, in_=bh_d)
            nc.scalar.dma_start(out=bo, in_=bo_d)
            for ti in groups[0]:
                front(ti, ramp=True)
            for ti in groups[0]:
                l0_mm(ti)
            for gi, group in enumerate(groups):
                for ti in group:
                    l0_relu(ti)
                nxt = groups[gi + 1] if gi + 1 < len(groups) else []
                for ti2 in nxt:
                    front_pre(ti2)
                for k in range(NL - 1):
                    for ti in group:
                        layer_mm(ti, k)
                    if k == 1:
                        for ti2 in nxt:
                            front_post(ti2)
                    if k == 2:
                        # out-layer h2 part fills the PE gap while the k2
                        # relu chain runs on ACT/DVE
                        for ti in group:
                            out_mm_h2(ti)
                    for ti in group:
                        layer_post(ti, k)
                for ti in group:
                    out_mm_t3(ti)
                for ti in nxt:
                    l0_mm(ti)
                for ti in group:
                    epilogue(ti)

    nc.compile()
    return nc


def _get_program():
    if "nc" not in _compiled:
        _compiled["nc"] = _build_program()
    return _compiled["nc"]


def _xe_perm():
    """perm[slot] = reference xe row for device slot order
    (slots: 0..3 = x', 4 + j*10 + i = sin_{j,i}, 44 + j*10 + i = cos)."""
    perm = np.zeros(84, np.int64)
    perm[0:4] = np.arange(4)
    for s in range(2):
        for j in range(4):
            for i in range(NUM_FREQS):
                perm[4 + s * 40 + j * 10 + i] = 4 + i * 8 + j * 2 + s
    return perm


def _prep_weights(e, W0, b0, Wh, bh, scal, Wout, bout):
    """Host-side layout transforms (permutation / reshape / cast only)."""
    bf = ml_dtypes.bfloat16
    w0 = np.zeros((128, HID), np.float32)
    w0[0:84] = W0[e][_xe_perm()]
    w0[84] = b0[e]
    w0 = w0.astype(bf)
    wh = np.ascontiguousarray(
        Wh[e].reshape(NL - 1, 2, 128, 2, 128)
        .transpose(2, 0, 1, 3, 4)).astype(bf)            # [128,3,2,2,128]
    wo = np.ascontiguousarray(
        Wout[e].reshape(2, 128, DOUT).transpose(1, 0, 2)).astype(bf)
    wos = np.ascontiguousarray(
        (scal[e, 2] * Wout[e]).reshape(2, 128, DOUT)
        .transpose(1, 0, 2)).astype(bf)
    bhr = np.ascontiguousarray(
        bh[e].reshape(NL - 1, 2, 128).transpose(2, 0, 1))  # [128,3,2]
    bor = np.ascontiguousarray(bout[e].reshape(DOUT, 1))
    sc3 = np.ascontiguousarray(scal[e])
    fr10 = (2.0 ** (np.arange(NUM_FREQS, dtype=np.float32) - 1.0)).astype(
        np.float32)
    return dict(w0=w0, wh=wh, wo=wo, wos=wos, bhr=bhr, bor=bor,
                scal3=sc3, fr10=fr10)


def kernel(x, in_dim, layer_id, W0, b0, Wh, bh, scal, Wout, bout):
    from concourse.bass_utils import run_bass_kernel_spmd

    x = np.asarray(x, np.float32)
    in_dim = np.asarray(in_dim, np.float32)
    layer_id = np.asarray(layer_id)
    W0 = np.asarray(W0, np.float32)
    b0 = np.asarray(b0, np.float32)
    Wh = np.asarray(Wh, np.float32)
    bh = np.asarray(bh, np.float32)
    scal = np.asarray(scal, np.float32)
    Wout = np.asarray(Wout, np.float32)
    bout = np.asarray(bout, np.float32)

    # ---- dispatch: expert e -> cores 2e, 2e+1; pad to CAP per core ----
    PADIDX = N
    x_aug = np.vstack([x, np.ones((1, 4), np.float32)])
    d_aug = np.concatenate([in_dim, np.ones(1, np.float32)])
    perms = np.full((NCORE, CAP), PADIDX, np.int64)
    overflow = []
    for e in range(E):
        idx = np.flatnonzero(layer_id == e)
        if len(idx) > 2 * CAP:
            overflow.append(idx[2 * CAP:])
            idx = idx[:2 * CAP]
        nh = min((len(idx) + 1) // 2, CAP)
        perms[2 * e, :nh] = idx[:nh]
        perms[2 * e + 1, :len(idx) - nh] = idx[nh:]

    in_maps = []
    for c in range(NCORE):
        m = _prep_weights(c // 2, W0, b0, Wh, bh, scal, Wout, bout)
        p = perms[c]
        # chunk-major: [...][cg, p] -> [p, cg]; x and in_dim packed
        xd = np.concatenate([x_aug[p], d_aug[p][:, None]], axis=1)
        m["xd_cm"] = np.ascontiguousarray(
            xd.reshape(CG, 128, 5).transpose(1, 0, 2))
        in_maps.append(m)

    nc = _get_program()
    res = run_bass_kernel_spmd(nc, in_maps, core_ids=list(range(NCORE)),
                               **RUN_KWARGS)
    LAST_RESULT.clear()
    LAST_RESULT.append(res)

    out = np.zeros((N + 1, DOUT), np.float32)
    for c in range(NCORE):
        r = np.asarray(res.results[c]["out_cm"], np.float32)
        out[perms[c]] = r.transpose(1, 0, 2).reshape(CAP, DOUT)

    # pathological overflow fallback (never hit for the benchmark input)
    if overflow:
        ov = np.concatenate(overflow)
        out[ov] = _numpy_ref(x[ov], in_dim[ov], layer_id[ov], W0, b0, Wh, bh,
                             scal, Wout, bout)
    return out[:N]


def _numpy_ref(x, in_dim, layer_id, W0, b0, Wh, bh, scal, Wout, bout):
    x = np.concatenate([x[:, :3] / x[:, 3:4], x[:, 3:]], axis=1)
    freqs = (2.0 ** np.arange(NUM_FREQS, dtype=np.float32)) * np.float32(np.pi)
    ang = x[:, None, :] * freqs[None, :, None]
    sc = np.stack([np.sin(ang), np.cos(ang)], axis=-1)
    xe = np.concatenate([x, sc.reshape(x.shape[0], -1)], axis=1)
    out = np.zeros((x.shape[0], DOUT), np.float32)
    for e in range(E):
        m = layer_id == e
        if not m.any():
            continue
        h = np.maximum(xe[m] @ W0[e] + b0[e], 0.0)
        for k in range(NL - 1):
            h = scal[e, k] * np.maximum(h @ Wh[e, k] + bh[e, k], 0.0) + h
        out[m] = h @ Wout[e] + bout[e]
    return out / in_dim[:, None]
